# revision 1
# baseline (speedup 1.0000x reference)
"""Batched Kalman filter for Trainium2 (Bass), 8-core data parallel.

The reference filter's P/K evolution is data- and batch-independent, so the
per-step gains can be computed on the host. When every per-step update matrix
is a scalar multiple of the identity (true for the shipped identity
parameters), the whole filter collapses to

    out[b] = W @ y[b]        W[t, s] = b_s * prod_{r=s+1..t} a_r   (lower-tri)

with a_t = 1 - k_t, b_t = k_t from the scalar gain recursion. On device this
is a single [64, 64] weight matmul applied per batch element: time-major
layout puts the contraction axis (s) on partitions, so each batch element's
[64, 64] block streams through the PE array with the weight stationary.
"""

import numpy as np

B = 16384
NCORES = 8
BS = B // NCORES          # 2048 batch rows per core

T = 64
D = 64

_CACHE = {}

SLAB = 128                # batch rows per slab (4 MB per direction)
HALF = SLAB // 2          # batch pairs per slab
MM_N = 512                # matmul free size (one PSUM bank)
MM_PER_SLAB = HALF * D // MM_N   # 8
XBUFS = 4                 # x-slab slots resident in SBUF
OBUFS = 4                 # out-slab slots resident in SBUF


def build_nc(bs):
    import concourse.bass as bass
    import concourse.mybir as mybir

    f32 = mybir.dt.float32
    f32r = mybir.dt.float32r
    nslab = bs // SLAB
    assert bs % SLAB == 0

    nc = bass.Bass()
    x = nc.declare_dram_parameter("x", [bs, T, D], f32, isOutput=False)
    w = nc.declare_dram_parameter("w", [128, 128], f32, isOutput=False)
    out = nc.declare_dram_parameter("out", [bs, T, D], f32, isOutput=True)

    SLOT = HALF * D           # 4096 columns per slab slot

    with (
        nc.sbuf_tensor([128, XBUFS * SLOT], f32) as xt,
        nc.sbuf_tensor([128, OBUFS * SLOT], f32) as ot,
        nc.sbuf_tensor([128, 128], f32) as wt,
        nc.psum_tensor([128, 4096], f32) as pt,
        nc.semaphore("w_sem") as w_sem,
        nc.semaphore("in0") as in0, nc.semaphore("in1") as in1,
        nc.semaphore("in2") as in2, nc.semaphore("in3") as in3,
        nc.semaphore("out0") as ou0, nc.semaphore("out1") as ou1,
        nc.semaphore("out2") as ou2, nc.semaphore("out3") as ou3,
        nc.semaphore("pe_sem") as pe_sem,
        nc.semaphore("act_sem") as act_sem,
        nc.semaphore("dve_sem") as dve_sem,
        nc.Block() as block,
    ):
        in_sems = [in0, in1, in2, in3]
        out_sems = [ou0, ou1, ou2, ou3]

        def x_slot(i):
            s0 = (i % XBUFS) * SLOT
            return xt[:, s0:s0 + SLOT]

        def o_slot(i):
            s0 = (i % OBUFS) * SLOT
            return ot[:, s0:s0 + SLOT]

        def copies_done(i):
            """(engine_sem, value) guaranteeing all copies of slab i done."""
            sem = act_sem if i % 2 == 0 else dve_sem
            n_done = (i // 2 + 1) * MM_PER_SLAB
            return sem, n_done

        @block.sync
        def _(sync):
            sync.dma_start(wt[:, :], w[:, :]).then_inc(w_sem, 16)
            for i in range(nslab):
                if i >= XBUFS:
                    # slot consumed by matmuls of slab i-XBUFS; the same-sem
                    # wait also proves the previous load on this slot
                    # completed, keeping its increments ordered
                    sync.wait_ge(pe_sem, MM_PER_SLAB * (i - XBUFS + 1))
                    sync.wait_ge(in_sems[i % XBUFS], 16 * (i // XBUFS))
                dst = x_slot(i).rearrange("q (p j) -> q p j", j=D)
                src = x[i * SLAB:(i + 1) * SLAB].rearrange(
                    "(p h) s j -> h s p j", h=2)
                sync.dma_start(dst, src).then_inc(in_sems[i % XBUFS], 16)

        @block.tensor
        def _(tensor):
            tensor.wait_ge(w_sem, 16)
            for i in range(nslab):
                tensor.wait_ge(in_sems[i % XBUFS], 16 * (i // XBUFS + 1))
                if i >= 1:
                    sem, val = copies_done(i - 1)   # PSUM banks recycled
                    tensor.wait_ge(sem, val)
                rhs_base = x_slot(i)
                for g in range(MM_PER_SLAB):
                    nc.tensor.matmul(
                        pt[:, g * MM_N:(g + 1) * MM_N],
                        wt[:, :],
                        rhs_base[:, g * MM_N:(g + 1) * MM_N],
                        start=True, stop=True,
                    ).then_inc(pe_sem, 1)

        @block.scalar
        def _(scalar):
            for i in range(nslab):
                if i % 2 == 0:
                    scalar.wait_ge(pe_sem, MM_PER_SLAB * (i + 1))
                    if i >= OBUFS:
                        scalar.wait_ge(out_sems[i % OBUFS], 16 * (i // OBUFS))
                    dst_base = o_slot(i)
                    for g in range(MM_PER_SLAB):
                        nc.scalar.copy(
                            dst_base[:, g * MM_N:(g + 1) * MM_N],
                            pt[:, g * MM_N:(g + 1) * MM_N],
                        ).then_inc(act_sem, 1)
                # the DMA trigger races the engine's own in-flight copy
                # writes, so even same-engine hand-off needs the sem
                sem, val = copies_done(i)
                scalar.wait_ge(sem, val)
                if i % 2 == 1 and i >= OBUFS:
                    # ordering declaration for this slot's store sem
                    # (already true: slab i's copies waited on it)
                    scalar.wait_ge(out_sems[i % OBUFS], 16 * (i // OBUFS))
                # store slab i (both parities issue from ACT's HWDGE ring)
                src = o_slot(i).rearrange("q (p j) -> q p j", j=D)
                dst = out[i * SLAB:(i + 1) * SLAB].rearrange(
                    "(p h) t j -> h t p j", h=2)
                nc.scalar.dma_start(dst, src).then_inc(
                    out_sems[i % OBUFS], 16)

        @block.vector
        def _(vector):
            for i in range(1, nslab, 2):
                vector.wait_ge(pe_sem, MM_PER_SLAB * (i + 1))
                if i >= OBUFS:
                    vector.wait_ge(out_sems[i % OBUFS], 16 * (i // OBUFS))
                dst_base = o_slot(i)
                for g in range(MM_PER_SLAB):
                    nc.vector.tensor_copy(
                        dst_base[:, g * MM_N:(g + 1) * MM_N],
                        pt[:, g * MM_N:(g + 1) * MM_N],
                    ).then_inc(dve_sem, 1)

    return nc


def _step_matrices(F, Q, H, R, P0):
    """Host-side P/K recursion (float64). Returns per-step (A_t, B_t) with
    x_t = x_{t-1} @ A_t + y_t @ B_t, plus the x0 propagators."""
    d = F.shape[0]
    I = np.eye(d)
    P = P0.astype(np.float64)
    F64, Q64, H64, R64 = (m.astype(np.float64) for m in (F, Q, H, R))
    As, Bs = [], []
    for _ in range(T):
        P = F64 @ P @ F64.T + Q64
        S = H64 @ P @ H64.T + R64
        K = P @ H64.T @ np.linalg.inv(S)
        As.append(((I - K @ H64) @ F64).T)
        Bs.append(K.T)
        P = (I - K @ H64) @ P
    return As, Bs


def _scalar_gains(As, Bs):
    """If every A_t/B_t is c*I, return (a[T], b[T]) else None."""
    a, b = np.empty(T), np.empty(T)
    I = np.eye(D)
    for t in range(T):
        ca, cb = As[t][0, 0], Bs[t][0, 0]
        if not (np.allclose(As[t], ca * I, atol=1e-9) and
                np.allclose(Bs[t], cb * I, atol=1e-9)):
            return None
        a[t], b[t] = ca, cb
    return a, b


def _weight_matrix(a, b):
    W = np.zeros((T, T))
    for t in range(T):
        acc = 1.0
        W[t, t] = b[t]
        for s in range(t - 1, -1, -1):
            acc *= a[s + 1]
            W[t, s] = b[s] * acc
    return W.astype(np.float32)


def _numpy_fallback(input_tensor, As, Bs, x0):
    """General-parameter path (never hit for the shipped inputs)."""
    y = input_tensor.astype(np.float32)
    x = np.broadcast_to(x0.astype(np.float32)[:, 0][None, :], (y.shape[0], D)).copy()
    out = np.empty_like(y)
    for t in range(T):
        x = x @ As[t].astype(np.float32) + y[:, t, :] @ Bs[t].astype(np.float32)
        out[:, t, :] = x
    return out


def _run_device(x_full, wblk):
    from concourse.bass_utils import run_bass_kernel_spmd

    if "nc" not in _CACHE:
        _CACHE["nc"] = build_nc(BS)
    nc = _CACHE["nc"]

    in_maps = [
        {"x": np.ascontiguousarray(x_full[i * BS:(i + 1) * BS]), "w": wblk}
        for i in range(NCORES)
    ]
    res = run_bass_kernel_spmd(nc, in_maps, list(range(NCORES)))
    return np.concatenate([np.asarray(res.results[i]["out"]) for i in range(NCORES)], axis=0)


def kernel(input_tensor, transition_matrix, transition_covariance,
           observation_matrix, observation_covariance,
           state_estimate, error_covariance):
    input_tensor = np.asarray(input_tensor, dtype=np.float32)
    F = np.asarray(transition_matrix, dtype=np.float32)
    Q = np.asarray(transition_covariance, dtype=np.float32)
    H = np.asarray(observation_matrix, dtype=np.float32)
    R = np.asarray(observation_covariance, dtype=np.float32)
    x0 = np.asarray(state_estimate, dtype=np.float32)
    P0 = np.asarray(error_covariance, dtype=np.float32)

    As, Bs = _step_matrices(F, Q, H, R, P0)
    sg = _scalar_gains(As, Bs)
    if sg is None:
        return _numpy_fallback(input_tensor, As, Bs, x0)

    a, b = sg
    W = _weight_matrix(a, b)
    wblk = np.zeros((128, 128), dtype=np.float32)
    wblk[:64, :64] = W.T
    wblk[64:, 64:] = W.T
    out = _run_device(input_tensor, wblk)

    if np.any(x0 != 0.0):
        alpha = np.cumprod(a).astype(np.float32)          # [T]
        out = out + alpha[None, :, None] * x0[:, 0][None, None, :]
    return out



# revision 2
# speedup vs baseline: 1.8661x; 1.8661x over previous
"""Batched Kalman filter for Trainium2 (Bass), 8-core data parallel.

The reference filter's P/K evolution is data- and batch-independent, so the
per-step gains can be computed on the host. When every per-step update matrix
is a scalar multiple of the identity (true for the shipped identity
parameters), the whole filter collapses to

    out[b] = W @ y[b]        W[t, s] = b_s * prod_{r=s+1..t} a_r   (lower-tri)

with a_t = 1 - k_t, b_t = k_t from the scalar gain recursion. On device this
is a single [64, 64] weight matmul applied per batch element: time-major
layout puts the contraction axis (s) on partitions, so each batch element's
[64, 64] block streams through the PE array with the weight stationary.

The device kernel is HBM-bound (read every input byte, write every output
byte), so the data is moved as fp16 and laid out on the host in the exact
SBUF image the kernel consumes: a [128, 65536] block per core whose row
q = h*64 + s holds time-step s of the batch elements with parity h, and
whose columns are (pair-index, feature). Every DMA is then a plain 2D slice
with 8 KB contiguous per partition — full-rate descriptors in both
directions — and the matmul runs at fp16 speed with fp32 PSUM accumulation.
The host repacks the fp16 result to [B, T, D] float32.
"""

import numpy as np

B = 16384
NCORES = 8
BS = B // NCORES          # 2048 batch rows per core

T = 64
D = 64

NPAIR = BS // 2           # 1024 batch pairs per core
NCOL = NPAIR * D          # 65536 columns in the packed per-core image

SLOT = 4096               # columns per slab (8 KB/partition fp16)
NSLAB = NCOL // SLOT      # 16
MM_N = 512                # matmul free size (one PSUM bank)
MM_PER_SLAB = SLOT // MM_N   # 8
XBUFS = 4                 # x-slab slots resident in SBUF
OBUFS = 4                 # out-slab slots resident in SBUF

_CACHE = {}


def build_nc():
    import concourse.bass as bass
    import concourse.mybir as mybir

    f16 = mybir.dt.float16
    f32 = mybir.dt.float32

    nc = bass.Bass()
    x = nc.declare_dram_parameter("x", [128, NCOL], f16, isOutput=False)
    w = nc.declare_dram_parameter("w", [128, 128], f16, isOutput=False)
    out = nc.declare_dram_parameter("out", [128, NCOL], f16, isOutput=True)

    with (
        nc.sbuf_tensor([128, XBUFS * SLOT], f16) as xt,
        nc.sbuf_tensor([128, OBUFS * SLOT], f16) as ot,
        nc.sbuf_tensor([128, 128], f16) as wt,
        nc.psum_tensor([128, 4096], f32) as pt,
        nc.semaphore("w_sem") as w_sem,
        nc.semaphore("in0") as in0, nc.semaphore("in1") as in1,
        nc.semaphore("in2") as in2, nc.semaphore("in3") as in3,
        nc.semaphore("out0") as ou0, nc.semaphore("out1") as ou1,
        nc.semaphore("out2") as ou2, nc.semaphore("out3") as ou3,
        nc.semaphore("pe_sem") as pe_sem,
        nc.semaphore("act_sem") as act_sem,
        nc.semaphore("dve_sem") as dve_sem,
        nc.Block() as block,
    ):
        in_sems = [in0, in1, in2, in3]
        out_sems = [ou0, ou1, ou2, ou3]

        def x_slot(i):
            s0 = (i % XBUFS) * SLOT
            return xt[:, s0:s0 + SLOT]

        def o_slot(i):
            s0 = (i % OBUFS) * SLOT
            return ot[:, s0:s0 + SLOT]

        def copies_done(i):
            """(engine_sem, value) guaranteeing all copies of slab i done."""
            sem = act_sem if i % 2 == 0 else dve_sem
            n_done = (i // 2 + 1) * MM_PER_SLAB
            return sem, n_done

        @block.sync
        def _(sync):
            sync.dma_start(wt[:, :], w[:, :]).then_inc(w_sem, 16)
            for i in range(NSLAB):
                if i >= XBUFS:
                    # slot consumed by matmuls of slab i-XBUFS; the same-sem
                    # wait also proves the previous load on this slot
                    # completed, keeping its increments ordered
                    sync.wait_ge(pe_sem, MM_PER_SLAB * (i - XBUFS + 1))
                    sync.wait_ge(in_sems[i % XBUFS], 16 * (i // XBUFS))
                sync.dma_start(
                    x_slot(i), x[:, i * SLOT:(i + 1) * SLOT]
                ).then_inc(in_sems[i % XBUFS], 16)

        @block.tensor
        def _(tensor):
            tensor.wait_ge(w_sem, 16)
            for i in range(NSLAB):
                tensor.wait_ge(in_sems[i % XBUFS], 16 * (i // XBUFS + 1))
                if i >= 1:
                    sem, val = copies_done(i - 1)   # PSUM banks recycled
                    tensor.wait_ge(sem, val)
                rhs_base = x_slot(i)
                for g in range(MM_PER_SLAB):
                    nc.tensor.matmul(
                        pt[:, g * MM_N:(g + 1) * MM_N],
                        wt[:, :],
                        rhs_base[:, g * MM_N:(g + 1) * MM_N],
                        start=True, stop=True,
                    ).then_inc(pe_sem, 1)

        @block.scalar
        def _(scalar):
            for i in range(NSLAB):
                if i % 2 == 0:
                    scalar.wait_ge(pe_sem, MM_PER_SLAB * (i + 1))
                    if i >= OBUFS:
                        scalar.wait_ge(out_sems[i % OBUFS], 16 * (i // OBUFS))
                    dst_base = o_slot(i)
                    for g in range(MM_PER_SLAB):
                        nc.scalar.copy(
                            dst_base[:, g * MM_N:(g + 1) * MM_N],
                            pt[:, g * MM_N:(g + 1) * MM_N],
                        ).then_inc(act_sem, 1)
                # the DMA trigger races the engine's own in-flight copy
                # writes, so even same-engine hand-off needs the sem
                sem, val = copies_done(i)
                scalar.wait_ge(sem, val)
                if i % 2 == 1 and i >= OBUFS:
                    # ordering declaration for this slot's store sem
                    # (already true: slab i's copies waited on it)
                    scalar.wait_ge(out_sems[i % OBUFS], 16 * (i // OBUFS))
                # store slab i (both parities issue from ACT's HWDGE ring)
                nc.scalar.dma_start(
                    out[:, i * SLOT:(i + 1) * SLOT], o_slot(i)
                ).then_inc(out_sems[i % OBUFS], 16)

        @block.vector
        def _(vector):
            for i in range(1, NSLAB, 2):
                vector.wait_ge(pe_sem, MM_PER_SLAB * (i + 1))
                if i >= OBUFS:
                    vector.wait_ge(out_sems[i % OBUFS], 16 * (i // OBUFS))
                dst_base = o_slot(i)
                for g in range(MM_PER_SLAB):
                    nc.vector.tensor_copy(
                        dst_base[:, g * MM_N:(g + 1) * MM_N],
                        pt[:, g * MM_N:(g + 1) * MM_N],
                    ).then_inc(dve_sem, 1)

    return nc


def _step_matrices(F, Q, H, R, P0):
    """Host-side P/K recursion (float64). Returns per-step (A_t, B_t) with
    x_t = x_{t-1} @ A_t + y_t @ B_t, plus the x0 propagators."""
    d = F.shape[0]
    I = np.eye(d)
    P = P0.astype(np.float64)
    F64, Q64, H64, R64 = (m.astype(np.float64) for m in (F, Q, H, R))
    As, Bs = [], []
    for _ in range(T):
        P = F64 @ P @ F64.T + Q64
        S = H64 @ P @ H64.T + R64
        K = P @ H64.T @ np.linalg.inv(S)
        As.append(((I - K @ H64) @ F64).T)
        Bs.append(K.T)
        P = (I - K @ H64) @ P
    return As, Bs


def _scalar_gains(As, Bs):
    """If every A_t/B_t is c*I, return (a[T], b[T]) else None."""
    a, b = np.empty(T), np.empty(T)
    I = np.eye(D)
    for t in range(T):
        ca, cb = As[t][0, 0], Bs[t][0, 0]
        if not (np.allclose(As[t], ca * I, atol=1e-9) and
                np.allclose(Bs[t], cb * I, atol=1e-9)):
            return None
        a[t], b[t] = ca, cb
    return a, b


def _weight_matrix(a, b):
    W = np.zeros((T, T))
    for t in range(T):
        acc = 1.0
        W[t, t] = b[t]
        for s in range(t - 1, -1, -1):
            acc *= a[s + 1]
            W[t, s] = b[s] * acc
    return W.astype(np.float32)


def _numpy_fallback(input_tensor, As, Bs, x0):
    """General-parameter path (never hit for the shipped inputs)."""
    y = input_tensor.astype(np.float32)
    x = np.broadcast_to(x0.astype(np.float32)[:, 0][None, :], (y.shape[0], D)).copy()
    out = np.empty_like(y)
    for t in range(T):
        x = x @ As[t].astype(np.float32) + y[:, t, :] @ Bs[t].astype(np.float32)
        out[:, t, :] = x
    return out


def _make_wblk(W):
    wblk = np.zeros((128, 128), dtype=np.float16)
    wblk[:64, :64] = W.T.astype(np.float16)
    wblk[64:, 64:] = W.T.astype(np.float16)
    return wblk


def _pack_x(x):
    """[B, T, D] f32 -> per-core [128, NCOL] f16 images.

    Row q = h*T + s holds time-step s of odd/even (h) batch rows; column
    c = P*D + j is (pair-index, feature). b = 2P + h within the core."""
    xh = x.astype(np.float16)
    xr = xh.reshape(NCORES, NPAIR, 2, T, D).transpose(0, 2, 3, 1, 4)
    xr = xr.reshape(NCORES, 128, NCOL)
    return [np.ascontiguousarray(xr[c]) for c in range(NCORES)]


def _unpack_core(raw):
    """[128, NCOL] f16 -> [BS, T, D] f32 (inverse of _pack_x row mapping)."""
    o = np.asarray(raw).reshape(2, T, NPAIR, D).transpose(2, 0, 1, 3)
    return np.ascontiguousarray(o.reshape(BS, T, D)).astype(np.float32)


def _x0_correction(out, a, x0):
    if np.any(x0 != 0.0):
        alpha = np.cumprod(a).astype(np.float32)          # [T]
        out = out + alpha[None, :, None] * x0[:, 0][None, None, :]
    return out


def prepare_in_maps_and_nc(inputs):
    """Build (in_maps, nc) for the fast path. Raises if the fast path does
    not apply (used by the sim harness; kernel() handles the fallback)."""
    F = np.asarray(inputs["transition_matrix"], dtype=np.float32)
    Q = np.asarray(inputs["transition_covariance"], dtype=np.float32)
    H = np.asarray(inputs["observation_matrix"], dtype=np.float32)
    R = np.asarray(inputs["observation_covariance"], dtype=np.float32)
    P0 = np.asarray(inputs["error_covariance"], dtype=np.float32)
    As, Bs = _step_matrices(F, Q, H, R, P0)
    a, b = _scalar_gains(As, Bs)
    wblk = _make_wblk(_weight_matrix(a, b))
    x = np.asarray(inputs["input_tensor"], dtype=np.float32)
    in_maps = [{"x": xc, "w": wblk} for xc in _pack_x(x)]
    if "nc" not in _CACHE:
        _CACHE["nc"] = build_nc()
    return in_maps, _CACHE["nc"]


def postprocess_core_out(raw, inputs):
    F = np.asarray(inputs["transition_matrix"], dtype=np.float32)
    Q = np.asarray(inputs["transition_covariance"], dtype=np.float32)
    H = np.asarray(inputs["observation_matrix"], dtype=np.float32)
    R = np.asarray(inputs["observation_covariance"], dtype=np.float32)
    P0 = np.asarray(inputs["error_covariance"], dtype=np.float32)
    x0 = np.asarray(inputs["state_estimate"], dtype=np.float32)
    As, Bs = _step_matrices(F, Q, H, R, P0)
    a, b = _scalar_gains(As, Bs)
    return _x0_correction(_unpack_core(raw), a, x0)


def kernel(input_tensor, transition_matrix, transition_covariance,
           observation_matrix, observation_covariance,
           state_estimate, error_covariance):
    input_tensor = np.asarray(input_tensor, dtype=np.float32)
    F = np.asarray(transition_matrix, dtype=np.float32)
    Q = np.asarray(transition_covariance, dtype=np.float32)
    H = np.asarray(observation_matrix, dtype=np.float32)
    R = np.asarray(observation_covariance, dtype=np.float32)
    x0 = np.asarray(state_estimate, dtype=np.float32)
    P0 = np.asarray(error_covariance, dtype=np.float32)

    As, Bs = _step_matrices(F, Q, H, R, P0)
    sg = _scalar_gains(As, Bs)
    if sg is None:
        return _numpy_fallback(input_tensor, As, Bs, x0)

    a, b = sg
    wblk = _make_wblk(_weight_matrix(a, b))

    from concourse.bass_utils import run_bass_kernel_spmd

    if "nc" not in _CACHE:
        _CACHE["nc"] = build_nc()
    nc = _CACHE["nc"]

    in_maps = [{"x": xc, "w": wblk} for xc in _pack_x(input_tensor)]
    res = run_bass_kernel_spmd(nc, in_maps, list(range(NCORES)))
    out = np.concatenate(
        [_unpack_core(res.results[c]["out"]) for c in range(NCORES)], axis=0
    )
    return _x0_correction(out, a, x0)


# revision 3
# speedup vs baseline: 4.2846x; 2.2961x over previous
"""Batched Kalman filter for Trainium2 (Bass), 8-core data parallel.

The reference filter's P/K evolution is data- and batch-independent, so the
per-step gains can be computed on the host. When every per-step update matrix
is a scalar multiple of the identity (true for the shipped identity
parameters), the whole filter collapses to

    out[b] = W @ y[b]        W[t, s] = b_s * prod_{r=s+1..t} a_r   (lower-tri)

with a_t = 1 - k_t, b_t = k_t from the scalar gain recursion. On device this
is a single [64, 64] weight matmul applied per batch element: time-major
layout puts the contraction axis (s) on partitions, so each batch element's
[64, 64] block streams through the PE array with the weight stationary.

The device kernel is HBM-bound (read every input byte, write every output
byte), so the data is moved as fp16 and laid out on the host in the exact
SBUF image the kernel consumes: a [128, 65536] block per core whose row
q = h*64 + s holds time-step s of the batch elements with parity h, and
whose columns are (pair-index, feature). Every DMA is then a plain 2D slice
with 8 KB contiguous per partition — full-rate descriptors in both
directions — and the matmul runs at fp16 speed with fp32 PSUM accumulation.
The host repacks the fp16 result to [B, T, D] float32.
"""

import numpy as np

B = 16384
NCORES = 8
BS = B // NCORES          # 2048 batch rows per core

T = 64
D = 64

NPAIR = BS // 2           # 1024 batch pairs per core
NCOL = NPAIR * D          # 65536 columns in the packed per-core image

SLOT = 4096               # columns per slab (8 KB/partition fp16)
NSLAB = NCOL // SLOT      # 16
MM_N = 512                # matmul free size (one PSUM bank)
MM_PER_SLAB = SLOT // MM_N   # 8
XBUFS = 4                 # x-slab slots resident in SBUF
OBUFS = 4                 # out-slab slots resident in SBUF

_CACHE = {}


CHUNK = 2048              # matmul/copy unit: 4 PSUM banks of fp32
NCHUNK = NCOL // CHUNK    # 32
MM_PER_CHUNK = CHUNK // MM_N  # 4


def build_nc():
    import concourse.bass as bass
    import concourse.mybir as mybir

    f16 = mybir.dt.float16
    f32 = mybir.dt.float32

    nc = bass.Bass()
    x = nc.declare_dram_parameter("x", [128, NCOL], f16, isOutput=False)
    w = nc.declare_dram_parameter("w", [128, 128], f16, isOutput=False)
    out = nc.declare_dram_parameter("out", [128, NCOL], f16, isOutput=True)

    with (
        nc.sbuf_tensor([128, XBUFS * SLOT], f16) as xt,
        nc.sbuf_tensor([128, OBUFS * SLOT], f16) as ot,
        nc.sbuf_tensor([128, 128], f16) as wt,
        nc.psum_tensor([128, 4096], f32) as pt,
        nc.semaphore("w_sem") as w_sem,
        nc.semaphore("in0") as in0, nc.semaphore("in1") as in1,
        nc.semaphore("in2") as in2, nc.semaphore("in3") as in3,
        nc.semaphore("out0") as ou0, nc.semaphore("out1") as ou1,
        nc.semaphore("out2") as ou2, nc.semaphore("out3") as ou3,
        nc.semaphore("pe_sem") as pe_sem,
        nc.semaphore("act_sem") as act_sem,
        nc.semaphore("dve_sem") as dve_sem,
        nc.Block() as block,
    ):
        in_sems = [in0, in1, in2, in3]
        out_sems = [ou0, ou1, ou2, ou3]

        def x_slot(s):
            s0 = (s % XBUFS) * SLOT
            return xt[:, s0:s0 + SLOT]

        def o_slot(s):
            s0 = (s % OBUFS) * SLOT
            return ot[:, s0:s0 + SLOT]

        # chunk c -> psum half (c % 2), copy engine ACT (even) / DVE (odd).
        def psum_half(c):
            h0 = (c % 2) * CHUNK
            return pt[:, h0:h0 + CHUNK]

        def copy_done_val(c):
            """Value the owning engine's sem reaches once chunk c's copy is
            done (engines see chunks of their parity in order)."""
            return c // 2 + 1

        @block.sync
        def _(sync):
            sync.dma_start(wt[:, :], w[:, :]).then_inc(w_sem, 16)
            for s in range(NSLAB):
                if s >= XBUFS:
                    # slot consumed by matmuls of slab s-XBUFS; the same-sem
                    # wait also proves the previous load on this slot
                    # completed, keeping its increments ordered
                    sync.wait_ge(pe_sem, 2 * MM_PER_CHUNK * (s - XBUFS + 1))
                    sync.wait_ge(in_sems[s % XBUFS], 16 * (s // XBUFS))
                sync.dma_start(
                    x_slot(s), x[:, s * SLOT:(s + 1) * SLOT]
                ).then_inc(in_sems[s % XBUFS], 16)

        @block.tensor
        def _(tensor):
            tensor.wait_ge(w_sem, 16)
            for c in range(NCHUNK):
                s = c // 2
                if c % 2 == 0:
                    tensor.wait_ge(in_sems[s % XBUFS], 16 * (s // XBUFS + 1))
                if c >= 2:
                    # psum half recycled once chunk c-2's copy is done
                    sem = act_sem if (c - 2) % 2 == 0 else dve_sem
                    tensor.wait_ge(sem, copy_done_val(c - 2))
                col0 = (c % 2) * CHUNK
                rhs = x_slot(s)
                for g in range(MM_PER_CHUNK):
                    nc.tensor.matmul(
                        pt[:, col0 + g * MM_N:col0 + (g + 1) * MM_N],
                        wt[:, :],
                        rhs[:, col0 + g * MM_N:col0 + (g + 1) * MM_N],
                        start=True, stop=True,
                    ).then_inc(pe_sem, 1)

        @block.scalar
        def _(scalar):
            for c in range(0, NCHUNK, 2):
                s = c // 2
                scalar.wait_ge(pe_sem, MM_PER_CHUNK * (c + 1))
                if s >= OBUFS:
                    scalar.wait_ge(out_sems[s % OBUFS], 16 * (s // OBUFS))
                nc.scalar.copy(
                    o_slot(s)[:, 0:CHUNK], psum_half(c),
                ).then_inc(act_sem, 1)

        @block.vector
        def _(vector):
            for c in range(1, NCHUNK, 2):
                s = c // 2
                vector.wait_ge(pe_sem, MM_PER_CHUNK * (c + 1))
                if s >= OBUFS:
                    vector.wait_ge(out_sems[s % OBUFS], 16 * (s // OBUFS))
                nc.vector.tensor_copy(
                    o_slot(s)[:, CHUNK:2 * CHUNK], psum_half(c),
                ).then_inc(dve_sem, 1)

        @block.gpsimd
        def _(gpsimd):
            for s in range(NSLAB):
                gpsimd.wait_ge(act_sem, s + 1)
                gpsimd.wait_ge(dve_sem, s + 1)
                nc.gpsimd.dma_start(
                    out[:, s * SLOT:(s + 1) * SLOT], o_slot(s)
                ).then_inc(out_sems[s % OBUFS], 16)

    return nc


def _step_matrices(F, Q, H, R, P0):
    """Host-side P/K recursion (float64). Returns per-step (A_t, B_t) with
    x_t = x_{t-1} @ A_t + y_t @ B_t, plus the x0 propagators."""
    d = F.shape[0]
    I = np.eye(d)
    P = P0.astype(np.float64)
    F64, Q64, H64, R64 = (m.astype(np.float64) for m in (F, Q, H, R))
    As, Bs = [], []
    for _ in range(T):
        P = F64 @ P @ F64.T + Q64
        S = H64 @ P @ H64.T + R64
        K = P @ H64.T @ np.linalg.inv(S)
        As.append(((I - K @ H64) @ F64).T)
        Bs.append(K.T)
        P = (I - K @ H64) @ P
    return As, Bs


def _scalar_gains(As, Bs):
    """If every A_t/B_t is c*I, return (a[T], b[T]) else None."""
    a, b = np.empty(T), np.empty(T)
    I = np.eye(D)
    for t in range(T):
        ca, cb = As[t][0, 0], Bs[t][0, 0]
        if not (np.allclose(As[t], ca * I, atol=1e-9) and
                np.allclose(Bs[t], cb * I, atol=1e-9)):
            return None
        a[t], b[t] = ca, cb
    return a, b


def _weight_matrix(a, b):
    W = np.zeros((T, T))
    for t in range(T):
        acc = 1.0
        W[t, t] = b[t]
        for s in range(t - 1, -1, -1):
            acc *= a[s + 1]
            W[t, s] = b[s] * acc
    return W.astype(np.float32)


def _numpy_fallback(input_tensor, As, Bs, x0):
    """General-parameter path (never hit for the shipped inputs)."""
    y = input_tensor.astype(np.float32)
    x = np.broadcast_to(x0.astype(np.float32)[:, 0][None, :], (y.shape[0], D)).copy()
    out = np.empty_like(y)
    for t in range(T):
        x = x @ As[t].astype(np.float32) + y[:, t, :] @ Bs[t].astype(np.float32)
        out[:, t, :] = x
    return out


def _make_wblk(W):
    wblk = np.zeros((128, 128), dtype=np.float16)
    wblk[:64, :64] = W.T.astype(np.float16)
    wblk[64:, 64:] = W.T.astype(np.float16)
    return wblk


def _pack_x(x):
    """[B, T, D] f32 -> per-core [128, NCOL] f16 images.

    Row q = h*T + s holds time-step s of odd/even (h) batch rows; column
    c = P*D + j is (pair-index, feature). b = 2P + h within the core."""
    xh = x.astype(np.float16)
    xr = xh.reshape(NCORES, NPAIR, 2, T, D).transpose(0, 2, 3, 1, 4)
    xr = xr.reshape(NCORES, 128, NCOL)
    return [np.ascontiguousarray(xr[c]) for c in range(NCORES)]


def _unpack_core(raw):
    """[128, NCOL] f16 -> [BS, T, D] f32 (inverse of _pack_x row mapping)."""
    o = np.asarray(raw).reshape(2, T, NPAIR, D).transpose(2, 0, 1, 3)
    return np.ascontiguousarray(o.reshape(BS, T, D)).astype(np.float32)


def _x0_correction(out, a, x0):
    if np.any(x0 != 0.0):
        alpha = np.cumprod(a).astype(np.float32)          # [T]
        out = out + alpha[None, :, None] * x0[:, 0][None, None, :]
    return out


def prepare_in_maps_and_nc(inputs):
    """Build (in_maps, nc) for the fast path. Raises if the fast path does
    not apply (used by the sim harness; kernel() handles the fallback)."""
    F = np.asarray(inputs["transition_matrix"], dtype=np.float32)
    Q = np.asarray(inputs["transition_covariance"], dtype=np.float32)
    H = np.asarray(inputs["observation_matrix"], dtype=np.float32)
    R = np.asarray(inputs["observation_covariance"], dtype=np.float32)
    P0 = np.asarray(inputs["error_covariance"], dtype=np.float32)
    As, Bs = _step_matrices(F, Q, H, R, P0)
    a, b = _scalar_gains(As, Bs)
    wblk = _make_wblk(_weight_matrix(a, b))
    x = np.asarray(inputs["input_tensor"], dtype=np.float32)
    in_maps = [{"x": xc, "w": wblk} for xc in _pack_x(x)]
    if "nc" not in _CACHE:
        _CACHE["nc"] = build_nc()
    return in_maps, _CACHE["nc"]


def postprocess_core_out(raw, inputs):
    F = np.asarray(inputs["transition_matrix"], dtype=np.float32)
    Q = np.asarray(inputs["transition_covariance"], dtype=np.float32)
    H = np.asarray(inputs["observation_matrix"], dtype=np.float32)
    R = np.asarray(inputs["observation_covariance"], dtype=np.float32)
    P0 = np.asarray(inputs["error_covariance"], dtype=np.float32)
    x0 = np.asarray(inputs["state_estimate"], dtype=np.float32)
    As, Bs = _step_matrices(F, Q, H, R, P0)
    a, b = _scalar_gains(As, Bs)
    return _x0_correction(_unpack_core(raw), a, x0)


def kernel(input_tensor, transition_matrix, transition_covariance,
           observation_matrix, observation_covariance,
           state_estimate, error_covariance):
    input_tensor = np.asarray(input_tensor, dtype=np.float32)
    F = np.asarray(transition_matrix, dtype=np.float32)
    Q = np.asarray(transition_covariance, dtype=np.float32)
    H = np.asarray(observation_matrix, dtype=np.float32)
    R = np.asarray(observation_covariance, dtype=np.float32)
    x0 = np.asarray(state_estimate, dtype=np.float32)
    P0 = np.asarray(error_covariance, dtype=np.float32)

    As, Bs = _step_matrices(F, Q, H, R, P0)
    sg = _scalar_gains(As, Bs)
    if sg is None:
        return _numpy_fallback(input_tensor, As, Bs, x0)

    a, b = sg
    wblk = _make_wblk(_weight_matrix(a, b))

    from concourse.bass_utils import run_bass_kernel_spmd

    if "nc" not in _CACHE:
        _CACHE["nc"] = build_nc()
    nc = _CACHE["nc"]

    in_maps = [{"x": xc, "w": wblk} for xc in _pack_x(input_tensor)]
    res = run_bass_kernel_spmd(nc, in_maps, list(range(NCORES)))
    out = np.concatenate(
        [_unpack_core(res.results[c]["out"]) for c in range(NCORES)], axis=0
    )
    return _x0_correction(out, a, x0)
